# revision 2
# baseline (speedup 1.0000x reference)
"""MaxUnpooling2D scatter-add kernel for Trainium2 (8 NeuronCores).

Reference semantics (per batch b):
    y = mask // (OW*C); x = (mask // C) % OW; f = channel index c
    out[b, y, x, c] += updates[b, h, w, c]      (duplicates sum)

Strategy (pure data-parallel over batch; 2 batches per core):
  - One-hot matmul routing per (plane c, q-group): psum[y, x] += A_q.T @ B_q
    where A_q[i, y] = onehot(Y_i) (stationary) and B_q[i, x] = onehot(X_i)*V_i
    (moving). PSUM accumulates the 32 q-groups of a plane; duplicates sum.
  - A-tiles: DVE is_equal against a materialized iota, [P, y, q] layout
    (q innermost keeps the 2x packed mode; the strided LDWEIGHTS this causes
    overlaps under the matmuls).
  - B-tiles: CONTIGUOUS [P, (q, x)] layout so the matmul moving stream runs
    at ~1 col/cycle (strided rhs measured 230 ns/pair vs 72 ns contiguous).
    Built two ways, split across engines for balance:
      * GPSIMD local_scatter (most planes): scatters V directly into the
        zeroed B-tile at idx = (q%8)*128 + X -- no is_equal, no mult.
      * DVE (every GP_MOD-th plane): xeq = is_equal (2x packed, strided
        layout) then a 1x transposing mult into the contiguous layout.
  - Evacuate psum[y, x] into PL[y, x, c] on ACT; one 8MB DMA per batch.
"""

import sys

sys.path.insert(0, "/opt/trn_rl_repo")

import numpy as np

import concourse.bacc as bacc
import concourse.tile as tile
from concourse import mybir, library_config
from concourse.bass_utils import run_bass_kernel_spmd

# Problem shape (hardcoded per contract)
B, H, W, C = 16, 64, 64, 128
OH, OW = 2 * H, 2 * W
N_CORES = 8
B_PER_CORE = B // N_CORES  # 2
HWF = H * W  # 4096
P = 128
Q = HWF // P  # 32 hw rows per partition
NCOL = Q * C  # 4096
NHALF = NCOL // 2  # decode in halves to save SBUF
QH = Q // 2

F32 = mybir.dt.float32
FP16 = mybir.dt.float16
I32 = mybir.dt.int32
I16 = mybir.dt.int16

# planes with (c % GP_MOD) >= GP_DVE go to the DVE B-path; rest GPSIMD
GP_MOD = 8
GP_DVE = 7  # c % 8 == 7 -> DVE plane (g = 7/8 on GPSIMD)


def build_nc(gp_mod=GP_MOD, gp_dve=GP_DVE):
    nc = bacc.Bacc("TRN2", target_bir_lowering=False, debug=False)

    upd = nc.declare_dram_parameter("updates", [B_PER_CORE, HWF, C], F32, isOutput=False)
    msk = nc.declare_dram_parameter("mask", [B_PER_CORE, HWF, C], I32, isOutput=False)
    iota_in = nc.declare_dram_parameter("iota", [P, P], F32, isOutput=False)
    jq_in = nc.declare_dram_parameter("jq", [P, Q], I32, isOutput=False)
    out = nc.declare_dram_parameter("out", [B_PER_CORE, OH, OW, C], F32, isOutput=True)

    def is_dve_plane(c):
        return (c % gp_mod) == gp_dve

    with tile.TileContext(nc) as tc:
        with (
            tc.tile_pool(name="const", bufs=1) as const_pool,
            tc.tile_pool(name="dec", bufs=1) as dec_pool,
            tc.tile_pool(name="tr", bufs=1) as tr_pool,
            tc.tile_pool(name="pl", bufs=1) as pl_pool,
            tc.tile_pool(name="apool", bufs=3) as a_pool,
            tc.tile_pool(name="bpool", bufs=3) as b_pool,
            tc.tile_pool(name="xpool", bufs=2) as x_pool,
            tc.tile_pool(name="psum", bufs=8, space="PSUM") as psum_pool,
        ):
            nc.gpsimd.load_library(library_config.local_scatter)

            iota_f = const_pool.tile([P, P], F32)
            nc.sync.dma_start(iota_f[:], iota_in[:])
            jq = const_pool.tile([P, Q], I32)
            nc.sync.dma_start(jq[:], jq_in[:])
            # iotaT[p, y, q] = y (fp16, innermost step 1 -> DVE 2x packed)
            iotaT = const_pool.tile([P, P, Q], FP16)
            nc.vector.tensor_copy(
                iotaT[:],
                iota_f[:].rearrange("p (y o) -> p y o", o=1).broadcast_to([P, P, Q]),
            )

            for b in range(B_PER_CORE):
                # ---- load + decode batch b (in column-halves) ----
                ytr = tr_pool.tile([P, C, Q], FP16, tag="ytr")
                vtr = tr_pool.tile([P, C, Q], FP16, tag="vtr")
                xtr = tr_pool.tile([P, C, Q], FP16, tag="xtr")
                idx16 = tr_pool.tile([P, C, Q], I16, tag="idx16")
                for h in range(2):
                    qs = slice(h * QH, (h + 1) * QH)
                    cs = slice(h * NHALF, (h + 1) * NHALF)
                    u_f = dec_pool.tile([P, NHALF], F32, tag="uf")
                    nc.sync.dma_start(
                        u_f[:], upd[b].rearrange("(p q) c -> p (q c)", p=P)[:, cs]
                    )
                    m = dec_pool.tile([P, NHALF], I32, tag="m")
                    nc.sync.dma_start(
                        m[:], msk[b].rearrange("(p q) c -> p (q c)", p=P)[:, cs]
                    )
                    nc.vector.tensor_copy(
                        vtr[:, :, qs], u_f[:].rearrange("p (q c) -> p c q", c=C)
                    )
                    yi = dec_pool.tile([P, NHALF], I32, tag="yi")
                    nc.vector.tensor_scalar(
                        yi[:], m[:], 14, None, mybir.AluOpType.logical_shift_right
                    )
                    nc.vector.tensor_copy(
                        ytr[:, :, qs], yi[:].rearrange("p (q c) -> p c q", c=C)
                    )
                    xi = dec_pool.tile([P, NHALF], I32, tag="xi")
                    nc.vector.tensor_scalar(
                        xi[:],
                        m[:],
                        7,
                        127,
                        mybir.AluOpType.logical_shift_right,
                        mybir.AluOpType.bitwise_and,
                    )
                    nc.vector.tensor_copy(
                        xtr[:, :, qs], xi[:].rearrange("p (q c) -> p c q", c=C)
                    )
                    # idx16[p, c, q] = X + (q%8)*128 (int16, for local_scatter)
                    nc.vector.scalar_tensor_tensor(
                        idx16[:, :, qs],
                        xi[:].rearrange("p (q c) -> p c q", c=C),
                        0,
                        jq[:, qs]
                        .rearrange("p (o q) -> p o q", o=1)
                        .broadcast_to([P, C, QH]),
                        mybir.AluOpType.add,
                        mybir.AluOpType.add,
                    )

                pl = pl_pool.tile([P, P, C], F32)  # [y, x, c]

                for c in range(C):
                    # stationary: a[p, y, q] = (iotaT == Y) -- DVE 2x packed
                    a_pl = a_pool.tile([P, P, Q], FP16, tag="a")
                    y_bc = (
                        ytr[:, c, :]
                        .rearrange("p (o q) -> p o q", o=1)
                        .broadcast_to([P, P, Q])
                    )
                    nc.vector.tensor_tensor(
                        a_pl[:], iotaT[:], y_bc, mybir.AluOpType.is_equal
                    )

                    # moving: bs[p, q*128 + x] = onehot(X)*V -- contiguous
                    bs = b_pool.tile([P, NCOL], FP16, tag="b")
                    if is_dve_plane(c):
                        xeq = x_pool.tile([P, P, Q], FP16, tag="xeq")
                        x_bc = (
                            xtr[:, c, :]
                            .rearrange("p (o q) -> p o q", o=1)
                            .broadcast_to([P, P, Q])
                        )
                        nc.vector.tensor_tensor(
                            xeq[:], iotaT[:], x_bc, mybir.AluOpType.is_equal
                        )
                        v_bc = (
                            vtr[:, c, :]
                            .rearrange("p (q o) -> p q o", o=1)
                            .broadcast_to([P, Q, P])
                        )
                        nc.vector.tensor_tensor(
                            bs[:].rearrange("p (q x) -> p q x", x=P),
                            xeq[:].rearrange("p x q -> p q x"),
                            v_bc,
                            mybir.AluOpType.mult,
                        )
                    else:
                        for g in range(4):
                            nc.gpsimd.local_scatter(
                                bs[:, g * 1024:(g + 1) * 1024],
                                vtr[:, c, g * 8:(g + 1) * 8],
                                idx16[:, c, g * 8:(g + 1) * 8],
                                channels=P,
                                num_elems=1024,
                                num_idxs=8,
                            )

                    acc = psum_pool.tile([P, P], F32)  # [y, x]
                    for q in range(Q):
                        nc.tensor.matmul(
                            acc[:],
                            a_pl[:, :, q],
                            bs[:, q * P:(q + 1) * P],
                            start=(q == 0),
                            stop=(q == Q - 1),
                        )
                    nc.scalar.copy(pl[:, :, c], acc[:])

                nc.sync.dma_start(out[b].rearrange("y x c -> y (x c)"), pl[:])

    nc.compile()
    return nc


_CACHED = {}


def _get_nc():
    if "nc" not in _CACHED:
        _CACHED["nc"] = build_nc()
    return _CACHED["nc"]


def make_in_maps(updates: np.ndarray, mask: np.ndarray):
    iota = np.broadcast_to(np.arange(P, dtype=np.float32), (P, P)).copy()
    jq = np.broadcast_to(
        ((np.arange(Q, dtype=np.int32) % 8) * 128), (P, Q)
    ).copy()
    in_maps = []
    for i in range(N_CORES):
        sl = slice(i * B_PER_CORE, (i + 1) * B_PER_CORE)
        in_maps.append(
            {
                "updates": np.ascontiguousarray(
                    updates[sl].reshape(B_PER_CORE, HWF, C), dtype=np.float32
                ),
                "mask": np.ascontiguousarray(
                    mask[sl].reshape(B_PER_CORE, HWF, C), dtype=np.int32
                ),
                "iota": iota,
                "jq": jq,
            }
        )
    return in_maps


def kernel(updates: np.ndarray, mask: np.ndarray) -> np.ndarray:
    nc = _get_nc()
    in_maps = make_in_maps(updates, mask)
    res = run_bass_kernel_spmd(nc, in_maps, list(range(N_CORES)))
    return np.concatenate([res.results[i]["out"] for i in range(N_CORES)], axis=0)


# revision 6
# speedup vs baseline: 1.0063x; 1.0063x over previous
"""MaxUnpooling2D scatter-add kernel for Trainium2 (8 NeuronCores).

Reference semantics (per batch b):
    y = mask // (OW*C); x = (mask // C) % OW; f = channel index c
    out[b, y, x, c] += updates[b, h, w, c]      (duplicates sum)

Strategy (pure data-parallel over batch; 2 batches per core):
  - One-hot matmul routing per (plane c, q-group): psum[y, x] += A_q.T @ B_q
    where A_q[i, y] = onehot(Y_i) (stationary) and B_q[i, x] = onehot(X_i)*V_i
    (moving). PSUM accumulates the 32 q-groups of a plane; duplicates sum.
  - A-tiles: DVE is_equal against a materialized iota, [P, y, q] layout
    (q innermost keeps the 2x packed mode; the strided LDWEIGHTS this causes
    overlaps under the matmuls).
  - B-tiles: CONTIGUOUS [P, (q, x)] layout so the matmul moving stream runs
    at ~1 col/cycle (strided rhs measured 230 ns/pair vs 72 ns contiguous).
    Built two ways, split across engines for balance:
      * GPSIMD local_scatter (most planes): scatters V directly into the
        zeroed B-tile at idx = (q%8)*128 + X -- no is_equal, no mult.
      * DVE (every GP_MOD-th plane): xeq = is_equal (2x packed, strided
        layout) then a 1x transposing mult into the contiguous layout.
  - Evacuate psum[y, x] into PL[y, x, c] on ACT; one 8MB DMA per batch.
"""

import sys

sys.path.insert(0, "/opt/trn_rl_repo")

import numpy as np

import concourse.bacc as bacc
import concourse.tile as tile
from concourse import mybir, library_config
from concourse.bass_utils import run_bass_kernel_spmd

# Problem shape (hardcoded per contract)
B, H, W, C = 16, 64, 64, 128
OH, OW = 2 * H, 2 * W
N_CORES = 8
B_PER_CORE = B // N_CORES  # 2
HWF = H * W  # 4096
P = 128
Q = HWF // P  # 32 hw rows per partition
NCOL = Q * C  # 4096
NHALF = NCOL // 2  # decode in halves to save SBUF
QH = Q // 2

F32 = mybir.dt.float32
FP16 = mybir.dt.float16
I32 = mybir.dt.int32
I16 = mybir.dt.int16

# planes with (c % GP_MOD) >= GP_DVE go to the DVE B-path; rest GPSIMD
GP_MOD = 8
GP_DVE = 7  # c % 8 == 7 -> DVE plane (g = 7/8 on GPSIMD)


def build_nc(gp_mod=GP_MOD, gp_dve=GP_DVE):
    nc = bacc.Bacc("TRN2", target_bir_lowering=False, debug=False)

    upd = nc.declare_dram_parameter("updates", [B_PER_CORE, HWF, C], F32, isOutput=False)
    msk = nc.declare_dram_parameter("mask", [B_PER_CORE, HWF, C], I32, isOutput=False)
    iota_in = nc.declare_dram_parameter("iota", [P, P], F32, isOutput=False)
    jq_in = nc.declare_dram_parameter("jq", [P, Q], I32, isOutput=False)
    out = nc.declare_dram_parameter("out", [B_PER_CORE, OH, OW, C], F32, isOutput=True)

    def is_dve_plane(c):
        return (c % gp_mod) == gp_dve

    with tile.TileContext(nc) as tc:
        with (
            tc.tile_pool(name="const", bufs=1) as const_pool,
            tc.tile_pool(name="dec", bufs=1) as dec_pool,
            tc.tile_pool(name="tr", bufs=1) as tr_pool,
            tc.tile_pool(name="pl", bufs=1) as pl_pool,
            tc.tile_pool(name="apool", bufs=3) as a_pool,
            tc.tile_pool(name="bpool", bufs=8) as b_pool,
            tc.tile_pool(name="xpool", bufs=2) as x_pool,
            tc.tile_pool(name="bdpool", bufs=1) as bd_pool,
            tc.tile_pool(name="psum", bufs=8, space="PSUM") as psum_pool,
        ):
            nc.gpsimd.load_library(library_config.local_scatter)

            iota_f = const_pool.tile([P, P], F32)
            nc.sync.dma_start(iota_f[:], iota_in[:])
            jq = const_pool.tile([P, Q], I32)
            nc.sync.dma_start(jq[:], jq_in[:])
            # iotaT[p, y, q] = y (fp16, innermost step 1 -> DVE 2x packed)
            iotaT = const_pool.tile([P, P, Q], FP16)
            nc.vector.tensor_copy(
                iotaT[:],
                iota_f[:].rearrange("p (y o) -> p y o", o=1).broadcast_to([P, P, Q]),
            )

            for b in range(B_PER_CORE):
                # ---- load + decode batch b (in column-halves) ----
                ytr = tr_pool.tile([P, C, Q], FP16, tag="ytr")
                vtr = tr_pool.tile([P, C, Q], FP16, tag="vtr")
                xtr = tr_pool.tile([P, C, Q], FP16, tag="xtr")
                idx16 = tr_pool.tile([P, C, Q], I16, tag="idx16")
                for h in range(2):
                    qs = slice(h * QH, (h + 1) * QH)
                    cs = slice(h * NHALF, (h + 1) * NHALF)
                    u_f = dec_pool.tile([P, NHALF], F32, tag="uf")
                    nc.sync.dma_start(
                        u_f[:], upd[b].rearrange("(p q) c -> p (q c)", p=P)[:, cs]
                    )
                    m = dec_pool.tile([P, NHALF], I32, tag="m")
                    nc.sync.dma_start(
                        m[:], msk[b].rearrange("(p q) c -> p (q c)", p=P)[:, cs]
                    )
                    nc.vector.tensor_copy(
                        vtr[:, :, qs], u_f[:].rearrange("p (q c) -> p c q", c=C)
                    )
                    yi = dec_pool.tile([P, NHALF], I32, tag="yi")
                    nc.vector.tensor_scalar(
                        yi[:], m[:], 14, None, mybir.AluOpType.logical_shift_right
                    )
                    nc.vector.tensor_copy(
                        ytr[:, :, qs], yi[:].rearrange("p (q c) -> p c q", c=C)
                    )
                    xi = dec_pool.tile([P, NHALF], I32, tag="xi")
                    nc.vector.tensor_scalar(
                        xi[:],
                        m[:],
                        7,
                        127,
                        mybir.AluOpType.logical_shift_right,
                        mybir.AluOpType.bitwise_and,
                    )
                    nc.vector.tensor_copy(
                        xtr[:, :, qs], xi[:].rearrange("p (q c) -> p c q", c=C)
                    )
                    # idx16[p, c, q] = X + (q%8)*128 (int16, for local_scatter)
                    nc.vector.scalar_tensor_tensor(
                        idx16[:, :, qs],
                        xi[:].rearrange("p (q c) -> p c q", c=C),
                        0,
                        jq[:, qs]
                        .rearrange("p (o q) -> p o q", o=1)
                        .broadcast_to([P, C, QH]),
                        mybir.AluOpType.add,
                        mybir.AluOpType.add,
                    )

                pl = pl_pool.tile([P, P, C], F32)  # [y, x, c]

                for c in range(C):
                    # stationary: a[p, y, q] = (iotaT == Y) -- DVE 2x packed
                    a_pl = a_pool.tile([P, P, Q], FP16, tag="a")
                    y_bc = (
                        ytr[:, c, :]
                        .rearrange("p (o q) -> p o q", o=1)
                        .broadcast_to([P, P, Q])
                    )
                    nc.vector.tensor_tensor(
                        a_pl[:], iotaT[:], y_bc, mybir.AluOpType.is_equal
                    )

                    # moving: bs[p, q*128 + x] = onehot(X)*V -- contiguous
                    if is_dve_plane(c):
                        bs = bd_pool.tile([P, NCOL], FP16, tag="bd")
                        xeq = x_pool.tile([P, P, Q], FP16, tag="xeq")
                        x_bc = (
                            xtr[:, c, :]
                            .rearrange("p (o q) -> p o q", o=1)
                            .broadcast_to([P, P, Q])
                        )
                        nc.vector.tensor_tensor(
                            xeq[:], iotaT[:], x_bc, mybir.AluOpType.is_equal
                        )
                        v_bc = (
                            vtr[:, c, :]
                            .rearrange("p (q o) -> p q o", o=1)
                            .broadcast_to([P, Q, P])
                        )
                        nc.vector.tensor_tensor(
                            bs[:].rearrange("p (q x) -> p q x", x=P),
                            xeq[:].rearrange("p x q -> p q x"),
                            v_bc,
                            mybir.AluOpType.mult,
                        )
                        chunks = [bs[:, g * 1024:(g + 1) * 1024] for g in range(4)]
                    else:
                        chunks = []
                        for g in range(4):
                            bc_t = b_pool.tile([P, 1024], FP16, tag="b")
                            nc.gpsimd.local_scatter(
                                bc_t[:],
                                vtr[:, c, g * 8:(g + 1) * 8],
                                idx16[:, c, g * 8:(g + 1) * 8],
                                channels=P,
                                num_elems=1024,
                                num_idxs=8,
                            )
                            chunks.append(bc_t[:])

                    acc = psum_pool.tile([P, P], F32)  # [y, x]
                    for q in range(Q):
                        nc.tensor.matmul(
                            acc[:],
                            a_pl[:, :, q],
                            chunks[q // 8][:, (q % 8) * P:(q % 8 + 1) * P],
                            start=(q == 0),
                            stop=(q == Q - 1),
                        )
                    nc.scalar.copy(pl[:, :, c], acc[:])
                    if c % 32 == 31:
                        c0 = c - 31
                        nc.sync.dma_start(
                            out[b][:, :, c0:c0 + 32], pl[:, :, c0:c0 + 32]
                        )

    nc.compile()
    return nc


_CACHED = {}


def _get_nc():
    if "nc" not in _CACHED:
        _CACHED["nc"] = build_nc()
    return _CACHED["nc"]


def make_in_maps(updates: np.ndarray, mask: np.ndarray):
    iota = np.broadcast_to(np.arange(P, dtype=np.float32), (P, P)).copy()
    jq = np.broadcast_to(
        ((np.arange(Q, dtype=np.int32) % 8) * 128), (P, Q)
    ).copy()
    in_maps = []
    for i in range(N_CORES):
        sl = slice(i * B_PER_CORE, (i + 1) * B_PER_CORE)
        in_maps.append(
            {
                "updates": np.ascontiguousarray(
                    updates[sl].reshape(B_PER_CORE, HWF, C), dtype=np.float32
                ),
                "mask": np.ascontiguousarray(
                    mask[sl].reshape(B_PER_CORE, HWF, C), dtype=np.int32
                ),
                "iota": iota,
                "jq": jq,
            }
        )
    return in_maps


def kernel(updates: np.ndarray, mask: np.ndarray) -> np.ndarray:
    nc = _get_nc()
    in_maps = make_in_maps(updates, mask)
    res = run_bass_kernel_spmd(nc, in_maps, list(range(N_CORES)))
    return np.concatenate([res.results[i]["out"] for i in range(N_CORES)], axis=0)


# revision 8
# speedup vs baseline: 1.0276x; 1.0211x over previous
"""MaxUnpooling2D scatter-add kernel for Trainium2 (8 NeuronCores).

Reference semantics (per batch b):
    y = mask // (OW*C); x = (mask // C) % OW; f = channel index c
    out[b, y, x, c] += updates[b, h, w, c]      (duplicates sum)

Strategy (pure data-parallel over batch; 2 batches per core):
  - One-hot matmul routing per (plane c, q-group): psum[y, x] += A_q.T @ B_q
    where A_q[i, y] = onehot(Y_i) (stationary) and B_q[i, x] = onehot(X_i)*V_i
    (moving). PSUM accumulates the 32 q-groups of a plane; duplicates sum.
  - A-tiles: DVE is_equal against a materialized iota, [P, y, q] layout
    (q innermost keeps the 2x packed mode; the strided LDWEIGHTS this causes
    overlaps under the matmuls).
  - B-tiles: CONTIGUOUS [P, (q, x)] layout so the matmul moving stream runs
    at ~1 col/cycle (strided rhs measured 230 ns/pair vs 72 ns contiguous).
    Built two ways, split across engines for balance:
      * GPSIMD local_scatter (most planes): scatters V directly into the
        zeroed B-tile at idx = (q%8)*128 + X -- no is_equal, no mult.
      * DVE (every GP_MOD-th plane): xeq = is_equal (2x packed, strided
        layout) then a 1x transposing mult into the contiguous layout.
  - Evacuate psum[y, x] into PL[y, x, c] on ACT; one 8MB DMA per batch.
"""

import sys

sys.path.insert(0, "/opt/trn_rl_repo")

import numpy as np

import concourse.bacc as bacc
import concourse.tile as tile
from concourse import mybir, library_config
from concourse.bass_utils import run_bass_kernel_spmd

# Problem shape (hardcoded per contract)
B, H, W, C = 16, 64, 64, 128
OH, OW = 2 * H, 2 * W
N_CORES = 8
B_PER_CORE = B // N_CORES  # 2
HWF = H * W  # 4096
P = 128
Q = HWF // P  # 32 hw rows per partition
NCOL = Q * C  # 4096
NHALF = NCOL // 2  # decode in halves to save SBUF
QH = Q // 2

F32 = mybir.dt.float32
FP16 = mybir.dt.float16
I32 = mybir.dt.int32
I16 = mybir.dt.int16

# planes with (c % GP_MOD) >= GP_DVE go to the DVE B-path; rest GPSIMD
GP_MOD = 8
GP_DVE = 7  # c % 8 == 7 -> DVE plane (g = 7/8 on GPSIMD)


def build_nc(gp_mod=GP_MOD, gp_dve=GP_DVE):
    nc = bacc.Bacc("TRN2", target_bir_lowering=False, debug=False)

    upd = nc.declare_dram_parameter("updates", [B_PER_CORE, HWF, C], F32, isOutput=False)
    msk = nc.declare_dram_parameter("mask", [B_PER_CORE, HWF, C], I32, isOutput=False)
    iota_in = nc.declare_dram_parameter("iota", [P, P], F32, isOutput=False)
    jq_in = nc.declare_dram_parameter("jq", [P, Q], I32, isOutput=False)
    out = nc.declare_dram_parameter("out", [B_PER_CORE, OH, OW, C], F32, isOutput=True)

    def is_dve_plane(c):
        return (c % gp_mod) == gp_dve

    with tile.TileContext(nc) as tc:
        with (
            tc.tile_pool(name="const", bufs=1) as const_pool,
            tc.tile_pool(name="dec", bufs=1) as dec_pool,
            tc.tile_pool(name="tr", bufs=1) as tr_pool,
            tc.tile_pool(name="pl", bufs=1) as pl_pool,
            tc.tile_pool(name="apool", bufs=3) as a_pool,
            tc.tile_pool(name="bpool", bufs=4) as b_pool,
            tc.tile_pool(name="bpool2", bufs=4) as b_pool2,
            tc.tile_pool(name="xpool", bufs=2) as x_pool,
            tc.tile_pool(name="bdpool", bufs=1) as bd_pool,
            tc.tile_pool(name="psum", bufs=8, space="PSUM") as psum_pool,
        ):
            nc.gpsimd.load_library(library_config.local_scatter)

            iota_f = const_pool.tile([P, P], F32)
            nc.sync.dma_start(iota_f[:], iota_in[:])
            jq = const_pool.tile([P, Q], I32)
            nc.sync.dma_start(jq[:], jq_in[:])
            # iotaT[p, y, q] = y (fp16, innermost step 1 -> DVE 2x packed)
            iotaT = const_pool.tile([P, P, Q], FP16)
            nc.vector.tensor_copy(
                iotaT[:],
                iota_f[:].rearrange("p (y o) -> p y o", o=1).broadcast_to([P, P, Q]),
            )

            for b in range(B_PER_CORE):
                # ---- load + decode batch b (in column-halves) ----
                ytr = tr_pool.tile([P, C, Q], FP16, tag="ytr")
                vtr = tr_pool.tile([P, C, Q], FP16, tag="vtr")
                xtr = tr_pool.tile([P, C, Q], FP16, tag="xtr")
                idx16 = tr_pool.tile([P, C, Q], I16, tag="idx16")
                for h in range(2):
                    qs = slice(h * QH, (h + 1) * QH)
                    cs = slice(h * NHALF, (h + 1) * NHALF)
                    u_f = dec_pool.tile([P, NHALF], F32, tag="uf")
                    nc.sync.dma_start(
                        u_f[:], upd[b].rearrange("(p q) c -> p (q c)", p=P)[:, cs]
                    )
                    m = dec_pool.tile([P, NHALF], I32, tag="m")
                    nc.sync.dma_start(
                        m[:], msk[b].rearrange("(p q) c -> p (q c)", p=P)[:, cs]
                    )
                    nc.vector.tensor_copy(
                        vtr[:, :, qs], u_f[:].rearrange("p (q c) -> p c q", c=C)
                    )
                    yi = dec_pool.tile([P, NHALF], I32, tag="yi")
                    nc.vector.tensor_scalar(
                        yi[:], m[:], 14, None, mybir.AluOpType.logical_shift_right
                    )
                    nc.vector.tensor_copy(
                        ytr[:, :, qs], yi[:].rearrange("p (q c) -> p c q", c=C)
                    )
                    xi = dec_pool.tile([P, NHALF], I32, tag="xi")
                    nc.vector.tensor_scalar(
                        xi[:],
                        m[:],
                        7,
                        127,
                        mybir.AluOpType.logical_shift_right,
                        mybir.AluOpType.bitwise_and,
                    )
                    nc.vector.tensor_copy(
                        xtr[:, :, qs], xi[:].rearrange("p (q c) -> p c q", c=C)
                    )
                    # idx16[p, c, q] = X + (q%8)*128 (int16, for local_scatter)
                    nc.vector.scalar_tensor_tensor(
                        idx16[:, :, qs],
                        xi[:].rearrange("p (q c) -> p c q", c=C),
                        0,
                        jq[:, qs]
                        .rearrange("p (o q) -> p o q", o=1)
                        .broadcast_to([P, C, QH]),
                        mybir.AluOpType.add,
                        mybir.AluOpType.add,
                    )

                pl = pl_pool.tile([P, P, C], F32)  # [y, x, c]

                for c in range(C):
                    # stationary: a[p, y, q] = (iotaT == Y) -- DVE 2x packed
                    a_pl = a_pool.tile([P, P, Q], FP16, tag="a")
                    y_bc = (
                        ytr[:, c, :]
                        .rearrange("p (o q) -> p o q", o=1)
                        .broadcast_to([P, P, Q])
                    )
                    nc.vector.tensor_tensor(
                        a_pl[:], iotaT[:], y_bc, mybir.AluOpType.is_equal
                    )

                    # moving: bs[p, q*128 + x] = onehot(X)*V -- contiguous
                    if is_dve_plane(c):
                        bs = bd_pool.tile([P, NCOL], FP16, tag="bd")
                        xeq = x_pool.tile([P, P, Q], FP16, tag="xeq")
                        x_bc = (
                            xtr[:, c, :]
                            .rearrange("p (o q) -> p o q", o=1)
                            .broadcast_to([P, P, Q])
                        )
                        nc.vector.tensor_tensor(
                            xeq[:], iotaT[:], x_bc, mybir.AluOpType.is_equal
                        )
                        v_bc = (
                            vtr[:, c, :]
                            .rearrange("p (q o) -> p q o", o=1)
                            .broadcast_to([P, Q, P])
                        )
                        nc.vector.tensor_tensor(
                            bs[:].rearrange("p (q x) -> p q x", x=P),
                            xeq[:].rearrange("p x q -> p q x"),
                            v_bc,
                            mybir.AluOpType.mult,
                        )
                        chunks = [bs[:, g * 1024:(g + 1) * 1024] for g in range(4)]
                    else:
                        chunks = []
                        for g in range(4):
                            bc_t = (b_pool if g % 2 == 0 else b_pool2).tile(
                                [P, 1024], FP16, tag="b"
                            )
                            nc.gpsimd.local_scatter(
                                bc_t[:],
                                vtr[:, c, g * 8:(g + 1) * 8],
                                idx16[:, c, g * 8:(g + 1) * 8],
                                channels=P,
                                num_elems=1024,
                                num_idxs=8,
                            )
                            chunks.append(bc_t[:])

                    acc = psum_pool.tile([P, P], F32)  # [y, x]
                    for q in range(Q):
                        nc.tensor.matmul(
                            acc[:],
                            a_pl[:, :, q],
                            chunks[q // 8][:, (q % 8) * P:(q % 8 + 1) * P],
                            start=(q == 0),
                            stop=(q == Q - 1),
                        )
                    nc.scalar.copy(pl[:, :, c], acc[:])
                    if c % 32 == 31:
                        c0 = c - 31
                        nc.sync.dma_start(
                            out[b][:, :, c0:c0 + 32], pl[:, :, c0:c0 + 32]
                        )

    nc.compile()
    return nc


_CACHED = {}


def _get_nc():
    if "nc" not in _CACHED:
        _CACHED["nc"] = build_nc()
    return _CACHED["nc"]


def make_in_maps(updates: np.ndarray, mask: np.ndarray):
    iota = np.broadcast_to(np.arange(P, dtype=np.float32), (P, P)).copy()
    jq = np.broadcast_to(
        ((np.arange(Q, dtype=np.int32) % 8) * 128), (P, Q)
    ).copy()
    in_maps = []
    for i in range(N_CORES):
        sl = slice(i * B_PER_CORE, (i + 1) * B_PER_CORE)
        in_maps.append(
            {
                "updates": np.ascontiguousarray(
                    updates[sl].reshape(B_PER_CORE, HWF, C), dtype=np.float32
                ),
                "mask": np.ascontiguousarray(
                    mask[sl].reshape(B_PER_CORE, HWF, C), dtype=np.int32
                ),
                "iota": iota,
                "jq": jq,
            }
        )
    return in_maps


def kernel(updates: np.ndarray, mask: np.ndarray) -> np.ndarray:
    nc = _get_nc()
    in_maps = make_in_maps(updates, mask)
    res = run_bass_kernel_spmd(nc, in_maps, list(range(N_CORES)))
    return np.concatenate([res.results[i]["out"] for i in range(N_CORES)], axis=0)
